# revision 32
# baseline (speedup 1.0000x reference)
"""Capsule-routing kernel for Trainium2, 8-core batch-parallel.

Reference computation (per example, In=4096, D=256, N=16, K=16, routings=3):
    u_hat = (x @ W).reshape(In, N, K)           # [In, 256] with m = n*16+k
    b = 0
    for j in range(3):
        c = softmax(b, axis=n)                   # [In, N]
        outputs = squash(sum_i c[i,n] u_hat[i,n,:])   # [N, K]
        if j < 2: b[i,n] = sum_k outputs[n,k] u_hat[i,n,k]

Key algebraic restructuring: u_hat is NEVER materialized.
  - outputs accumulation:  acc = (c^T x) W = yT^T @ W  with
    yT[d,n] = sum_i x[i,d] c[i,n]  (64 matmuls of 16-free per round)
  - b update:  b = (x W) S = x @ (W S)  with WS [256,16] built by 4 tiny
    matmuls from S = (masked outputs)^T scaled by rinv.
  - round 0 (c uniform 1/16): yT0 = colsum(x)/16 broadcast, where
    colsum accumulates via 1-col matmuls folded into the transpose phase.
  - squash: om = acc*mask is transposed on PE; Square runs on the
    128-partition form (32-free); nrm2 via PE matmul with ones;
    rinv = exp(-0.5*ln(nrm2+eps)) so ScalarE stays on ONE ACT table
    (natural_log_exp_and_others: Copy/Square/Exp/Ln).
x is shipped bf16 i-major; xT (lhsT for the b update) is built on
device with PE transposes. Everything runs in bf16 (tol 2e-2).
"""

import sys
from contextlib import ExitStack

sys.path.insert(0, "/opt/trn_rl_repo")

import numpy as np
import ml_dtypes

import concourse.bass as bass
import concourse.mybir as mybir
import concourse.tile as tile
from concourse import bacc
from concourse.bass_utils import run_bass_kernel_spmd

# All ScalarE funcs used here (Copy/Square/Exp/Ln) live together in the
# natural_log_exp_and_others ACT table. Put it first so the act-table
# insertion pass resolves every activation to that one table instead of
# thrashing between exp_and_others and natural_log (1283 ns per reload).
from concourse.hw_specs import get_activation_tables as _gat_orig


def _gat_pref(arch):
    t = _gat_orig(arch)
    pref = "natural_log_exp_and_others"
    if pref in t:
        return {pref: t[pref],
                **{k: v for k, v in t.items() if k != pref}}
    return t


bacc.get_activation_tables = _gat_pref

F32 = mybir.dt.float32
BF16 = mybir.dt.bfloat16
AF = mybir.ActivationFunctionType

N_CORES = 8
B = 32
IN = 4096
D = 256
N = 16
K = 16
M = N * K  # 256
EPS = 1e-7
N_EX = 4
N_T = 32  # i tiles of 128


def build_kernel():
    nc = bacc.Bacc("TRN2", target_bir_lowering=False, debug=False,
                   num_devices=N_CORES)

    x_d = nc.dram_tensor("x", [N_EX, 128, N_T, D], BF16, kind="ExternalInput")
    Wt_d = nc.dram_tensor("Wt", [128, 2, M], BF16, kind="ExternalInput")
    WtT_d = nc.dram_tensor("WtT", [128, 2, D], BF16, kind="ExternalInput")
    id128_d = nc.dram_tensor("id128", [128, 128], BF16, kind="ExternalInput")
    ones_d = nc.dram_tensor("ones128", [128, 1], BF16, kind="ExternalInput")
    bmask_d = nc.dram_tensor("bmask", [N, M], BF16, kind="ExternalInput")
    bmaskT_d = nc.dram_tensor("bmaskT", [128, 2, N], BF16,
                              kind="ExternalInput")
    out_d = nc.dram_tensor("out", [N_EX, N, K], F32, kind="ExternalOutput")

    with tile.TileContext(nc) as tc, ExitStack() as ctx:
        const_pool = ctx.enter_context(tc.tile_pool(name="consts", bufs=1))
        x_pool = ctx.enter_context(tc.tile_pool(name="x", bufs=4))
        xT_pool = ctx.enter_context(tc.tile_pool(name="xT", bufs=3))
        c_pool = ctx.enter_context(tc.tile_pool(name="c", bufs=4))
        small_pool = ctx.enter_context(tc.tile_pool(name="small", bufs=4))

        ps_t = ctx.enter_context(tc.tile_pool(name="ps_t", bufs=2, space="PSUM"))
        ps_b = ctx.enter_context(tc.tile_pool(name="ps_b", bufs=2, space="PSUM"))
        ps_m = ctx.enter_context(tc.tile_pool(name="ps_m", bufs=4, space="PSUM"))

        # ---- constants ----
        Wt = const_pool.tile([128, 2, M], BF16, tag="Wt")
        nc.sync.dma_start(Wt[:], Wt_d[:])
        WtT = const_pool.tile([128, 2, D], BF16, tag="WtT")
        nc.sync.dma_start(WtT[:], WtT_d[:])
        id128 = const_pool.tile([128, 128], BF16, tag="id128")
        nc.sync.dma_start(id128[:], id128_d[:])
        ones128 = const_pool.tile([128, 1], BF16, tag="ones128")
        nc.sync.dma_start(ones128[:], ones_d[:])
        onesrow = const_pool.tile([1, 128], BF16, tag="onesrow")
        nc.sync.dma_start(onesrow[:], ones_d.ap().rearrange("p o -> o p"))
        bmask = const_pool.tile([N, M], BF16, tag="bmask")
        nc.sync.dma_start(bmask[:], bmask_d[:])
        bmaskT = const_pool.tile([128, 2, N], BF16, tag="bmaskT")
        nc.sync.dma_start(bmaskT[:], bmaskT_d[:])
        out_stage = const_pool.tile([N, N_EX, K], F32, tag="out_stage")
        eps_t = const_pool.tile([N, 1], F32, tag="eps")
        nc.vector.memset(eps_t[:], EPS)
        eps_r = const_pool.tile([1, 1], F32, tag="eps_r")
        nc.vector.memset(eps_r[:], EPS)

        # ---- x loads (prefetch, chunked so compute can start early) ----
        x_tiles = []
        for e in range(N_EX):
            xs = x_pool.tile([128, N_T, D], BF16, tag="x")
            for q in range(4):
                nc.sync.dma_start(xs[:, 8 * q:8 * (q + 1), :],
                                  x_d[e, :, 8 * q:8 * (q + 1), :])
            x_tiles.append(xs)

        st = [dict() for _ in range(N_EX)]

        # one shared PSUM bank subdivided for the small per-round tiles.
        # layout (f32 columns): yT @0..32, accT @32..64, rbc @64..96,
        # wsps @96..128, nrow @128..144, acc(j2 only) @224..480 (parts 0..15)
        def misc_views():
            m = ps_m.tile([128, 480], F32, tag="misc")
            yT_ps = m[:, 0:32].rearrange("p (c n) -> p c n", n=N)
            accT_ps = m[:, 32:64].rearrange("p (c n) -> p c n", n=N)
            rbc_ps = m[:, 64:96].rearrange("p (c n) -> p c n", n=N)
            wsps = m[:, 96:128].rearrange("p (c n) -> p c n", n=N)
            nrow_ps = m[0:1, 128:144]
            acc_ps = m[0:N, 224:480]
            return yT_ps, accT_ps, rbc_ps, wsps, nrow_ps, acc_ps

        # big-copy engine rotation: P7 V5 A4 per 16
        cp_engines = [nc.gpsimd, nc.vector, nc.scalar, nc.gpsimd,
                      nc.vector, nc.gpsimd, nc.scalar, nc.vector,
                      nc.gpsimd, nc.gpsimd, nc.vector, nc.scalar,
                      nc.gpsimd, nc.vector, nc.gpsimd, nc.scalar]

        def phase_T(e):
            """Build xT[d, i] via PE transposes; init c_all to 1/16."""
            xs = x_tiles[e]
            xT = xT_pool.tile([128, 2, IN], BF16, tag="xT")
            c0 = c_pool.tile([128, N_T, N], BF16, tag="c_all")
            nc.gpsimd.memset(c0[:], 1.0 / N)
            st[e]["c_all"] = c0
            for tp in range(N_T // 2):
                psT = ps_t.tile([128, 2, 2, 128], BF16, tag="psT")
                for ti in range(2):
                    t = 2 * tp + ti
                    for dc in range(2):
                        nc.tensor.transpose(
                            psT[:, dc, ti, :],
                            xs[:, t, 128 * dc:128 * (dc + 1)], id128[:])
                eng = cp_engines[tp]
                dst = xT[:, :, 256 * tp:256 * (tp + 1)]
                if eng is nc.vector:
                    eng.tensor_copy(dst, psT[:])
                elif eng is nc.scalar:
                    eng.copy(dst.bitcast(F32), psT[:].bitcast(F32))
                else:
                    eng.tensor_copy(dst.bitcast(F32), psT[:].bitcast(F32))
            st[e]["xT"] = xT

        def phase_J(e, j):
            """One routing round: yT -> accT -> squash -> WS -> b ->
            softmax (j<2) or final output (j=2)."""
            xT = st[e]["xT"]
            xs = x_tiles[e]
            c_all = st[e]["c_all"]
            yT_ps, accT_ps, rbc_ps, wsps, nrow_ps, acc_ps = misc_views()

            # --- yT[d, n] = sum_i x[i, d] c[i, n] ---
            for t in range(N_T):
                for dc in range(2):
                    nc.tensor.matmul(
                        yT_ps[:, dc, :],
                        xs[:, t, 128 * dc:128 * (dc + 1)], c_all[:, t, :],
                        start=(t == 0), stop=(t == N_T - 1),
                        skip_group_check=True)
            yT_sb = small_pool.tile([128, 2, N], BF16, tag="yTsb")
            nc.gpsimd.tensor_copy(yT_sb[:], yT_ps[:])

            if j == 2:
                # final round: plain acc [16, 256], compact extraction
                for dc in range(2):
                    nc.tensor.matmul(acc_ps[:], yT_sb[:, dc, :], Wt[:, dc, :],
                                     start=(dc == 0), stop=(dc == 1),
                                     skip_group_check=True)
                om = small_pool.tile([N, M], BF16, tag="om")
                nc.gpsimd.tensor_mul(om[:], acc_ps[:], bmask[:])
                sq = small_pool.tile([N, M], BF16, tag="sq")
                nrm2 = small_pool.tile([N, 1], F32, tag="nrm2")
                nc.scalar.activation(sq[:], om[:], AF.Square,
                                     accum_out=nrm2[:])
                lnv = small_pool.tile([N, 1], F32, tag="lnv")
                nc.scalar.activation(lnv[:], nrm2[:], AF.Ln, bias=eps_t[:])
                rinv = small_pool.tile([N, 1], F32, tag="rinv")
                nc.scalar.activation(rinv[:], lnv[:], AF.Exp, scale=-0.5)
                o_c = small_pool.tile([N, K], F32, tag="o_c")
                nc.vector.tensor_reduce(
                    o_c[:], om[:].rearrange("p (g k) -> p k g", k=K),
                    axis=mybir.AxisListType.X, op=mybir.AluOpType.add)
                nc.vector.tensor_scalar_mul(out_stage[:, e, :], o_c[:],
                                            rinv[:])
                return

            # --- accT[m, n] = sum_d W[d, m] yT[d, n] (transposed form) ---
            for mc in range(2):
                for dc in range(2):
                    nc.tensor.matmul(
                        accT_ps[:, mc, :],
                        Wt[:, dc, 128 * mc:128 * (mc + 1)], yT_sb[:, dc, :],
                        start=(dc == 0), stop=(dc == 1),
                        skip_group_check=True)
            omT = small_pool.tile([128, 2, N], BF16, tag="omT")
            nc.gpsimd.tensor_mul(omT[:], accT_ps[:], bmaskT[:])

            # branch 1 (rinv): sqT -> nrow -> ln -> exp -> rbc
            sqT = small_pool.tile([128, 2, N], BF16, tag="sqT")
            nc.vector.tensor_mul(sqT[:], omT[:], omT[:])
            for mc in range(2):
                nc.tensor.matmul(nrow_ps[:], ones128[:], sqT[:, mc, :],
                                 start=(mc == 0), stop=(mc == 1),
                                 skip_group_check=True)
            lnr = small_pool.tile([1, N], F32, tag="lnr")
            nc.scalar.activation(lnr[:], nrow_ps[:], AF.Ln, bias=eps_r[:])
            rrow = small_pool.tile([1, N], BF16, tag="rrow")
            nc.scalar.activation(rrow[:], lnr[:], AF.Exp, scale=-0.5)
            for mc in range(2):
                nc.tensor.matmul(rbc_ps[:, mc, :], onesrow[:], rrow[:],
                                 start=True, stop=True,
                                 skip_group_check=True)

            # branch 2 (runs in parallel): WS_raw = W @ omT; rinv scales
            # out of the m-contraction, applied at the ws copy
            for dc in range(2):
                for mc in range(2):
                    nc.tensor.matmul(
                        wsps[:, dc, :],
                        WtT[:, mc, 128 * dc:128 * (dc + 1)], omT[:, mc, :],
                        start=(mc == 0), stop=(mc == 1),
                        skip_group_check=True)
            ws = small_pool.tile([128, 2, N], BF16, tag="ws")
            nc.vector.tensor_mul(ws[:], wsps[:], rbc_ps[:])

            # --- b = x @ WS ---
            b_ps = ps_b.tile([128, N_T, N], F32, tag="b")
            for t in range(N_T):
                for dc in range(2):
                    nc.tensor.matmul(
                        b_ps[:, t, :],
                        xT[:, dc, 128 * t:128 * (t + 1)], ws[:, dc, :],
                        start=(dc == 0), stop=(dc == 1),
                        skip_group_check=True)

            # --- softmax over n ---
            e_all = c_pool.tile([128, N_T, N], BF16, tag="e_all")
            nc.scalar.activation(e_all[:], b_ps[:], AF.Exp)
            s_sum = c_pool.tile([128, N_T], F32, tag="s_sum")
            nc.vector.tensor_reduce(s_sum[:], e_all[:],
                                    axis=mybir.AxisListType.X,
                                    op=mybir.AluOpType.add)
            s_r = c_pool.tile([128, N_T], F32, tag="s_r")
            nc.vector.reciprocal(s_r[:], s_sum[:])
            c_all = c_pool.tile([128, N_T, N], BF16, tag="c_all")
            nc.vector.tensor_mul(c_all[:], e_all[:],
                                 s_r[:].to_broadcast([128, N_T, N]))
            st[e]["c_all"] = c_all

        # wavefront over examples to keep PE dense across dependency stalls
        order = [(0, "T"), (0, 0), (1, "T"), (0, 1), (1, 0), (2, "T"),
                 (0, 2), (1, 1), (2, 0), (3, "T"), (1, 2), (2, 1), (3, 0),
                 (2, 2), (3, 1), (3, 2)]
        for e, ph in order:
            if ph == "T":
                phase_T(e)
            else:
                phase_J(e, ph)

        nc.sync.dma_start(out_d.ap().rearrange("e n k -> n e k"), out_stage[:])

    nc.compile()
    return nc


_NC_CACHE = {}


def _get_nc():
    if "nc" not in _NC_CACHE:
        _NC_CACHE["nc"] = build_kernel()
    return _NC_CACHE["nc"]


def make_const_inputs():
    bf = ml_dtypes.bfloat16
    id128 = np.eye(128, dtype=bf)
    ones128 = np.ones((128, 1), dtype=bf)
    bmask = np.zeros((N, M), dtype=np.float32)
    for n in range(N):
        bmask[n, n * K:(n + 1) * K] = 1.0
    # bmaskT[p, mc, n] = bmask[n, mc*128 + p]
    bmaskT = np.ascontiguousarray(
        bmask.T.reshape(2, 128, N).transpose(1, 0, 2))
    return id128, ones128, bmask.astype(bf), bmaskT.astype(bf)


def kernel(x, W, num_capsule=None, dim_capsule=None, routings=None, **_):
    bf = ml_dtypes.bfloat16
    x = np.asarray(x, dtype=np.float32)
    W = np.asarray(W, dtype=np.float32)
    assert x.shape == (B, IN, D), x.shape

    nc = _get_nc()
    id128, ones128, bmask, bmaskT = make_const_inputs()
    W0 = W[0]
    Wt = np.ascontiguousarray(
        W0.reshape(2, 128, M).transpose(1, 0, 2)).astype(bf)
    WtT = np.ascontiguousarray(
        W0.T.reshape(2, 128, D).transpose(1, 0, 2)).astype(bf)

    # x[b, i, d] -> [core, e, p, t, d] with i = t*128 + p
    xr = np.ascontiguousarray(
        x.reshape(N_CORES, N_EX, N_T, 128, D).transpose(0, 1, 3, 2, 4)
    ).astype(bf)

    in_maps = []
    for c in range(N_CORES):
        in_maps.append({"x": xr[c], "Wt": Wt, "WtT": WtT, "id128": id128,
                        "ones128": ones128, "bmask": bmask,
                        "bmaskT": bmaskT})

    res = run_bass_kernel_spmd(nc, in_maps, core_ids=list(range(N_CORES)))
    out = np.concatenate([r["out"] for r in res.results], axis=0)
    return out.astype(np.float32)


# revision 36
# speedup vs baseline: 1.0441x; 1.0441x over previous
"""Capsule-routing kernel for Trainium2, 8-core batch-parallel.

Reference computation (per example, In=4096, D=256, N=16, K=16, routings=3):
    u_hat = (x @ W).reshape(In, N, K)           # [In, 256] with m = n*16+k
    b = 0
    for j in range(3):
        c = softmax(b, axis=n)                   # [In, N]
        outputs = squash(sum_i c[i,n] u_hat[i,n,:])   # [N, K]
        if j < 2: b[i,n] = sum_k outputs[n,k] u_hat[i,n,k]

Key algebraic restructuring: u_hat is NEVER materialized.
  - outputs accumulation:  acc = (c^T x) W = yT^T @ W  with
    yT[d,n] = sum_i x[i,d] c[i,n]  (64 matmuls of 16-free per round)
  - b update:  b = (x W) S = x @ (W S)  with WS [256,16] built by 4 tiny
    matmuls from S = (masked outputs)^T scaled by rinv.
  - round 0 (c uniform 1/16): yT0 = colsum(x)/16 broadcast, where
    colsum accumulates via 1-col matmuls folded into the transpose phase.
  - squash: om = acc*mask is transposed on PE; Square runs on the
    128-partition form (32-free); nrm2 via PE matmul with ones;
    rinv = exp(-0.5*ln(nrm2+eps)) so ScalarE stays on ONE ACT table
    (natural_log_exp_and_others: Copy/Square/Exp/Ln).
x is shipped bf16 i-major; xT (lhsT for the b update) is built on
device with PE transposes. Everything runs in bf16 (tol 2e-2).
"""

import sys
from contextlib import ExitStack

sys.path.insert(0, "/opt/trn_rl_repo")

import numpy as np
import ml_dtypes

import concourse.bass as bass
import concourse.mybir as mybir
import concourse.tile as tile
from concourse import bacc
from concourse.bass_utils import run_bass_kernel_spmd

# All ScalarE funcs used here (Copy/Square/Exp/Ln) live together in the
# natural_log_exp_and_others ACT table. Put it first so the act-table
# insertion pass resolves every activation to that one table instead of
# thrashing between exp_and_others and natural_log (1283 ns per reload).
from concourse.hw_specs import get_activation_tables as _gat_orig


def _gat_pref(arch):
    t = _gat_orig(arch)
    pref = "natural_log_exp_and_others"
    if pref in t:
        return {pref: t[pref],
                **{k: v for k, v in t.items() if k != pref}}
    return t


bacc.get_activation_tables = _gat_pref

F32 = mybir.dt.float32
BF16 = mybir.dt.bfloat16
AF = mybir.ActivationFunctionType

N_CORES = 8
B = 32
IN = 4096
D = 256
N = 16
K = 16
M = N * K  # 256
EPS = 1e-7
N_EX = 4
N_T = 32  # i tiles of 128


def build_kernel():
    nc = bacc.Bacc("TRN2", target_bir_lowering=False, debug=False,
                   num_devices=N_CORES)

    x_d = nc.dram_tensor("x", [N_EX, 128, N_T, D], BF16, kind="ExternalInput")
    Wt_d = nc.dram_tensor("Wt", [128, 2, M], BF16, kind="ExternalInput")
    WtT_d = nc.dram_tensor("WtT", [128, 2, D], BF16, kind="ExternalInput")
    id128_d = nc.dram_tensor("id128", [128, 128], BF16, kind="ExternalInput")
    ones_d = nc.dram_tensor("ones128", [128, 1], BF16, kind="ExternalInput")
    bmask_d = nc.dram_tensor("bmask", [N, M], BF16, kind="ExternalInput")
    bmaskT_d = nc.dram_tensor("bmaskT", [128, 2, N], BF16,
                              kind="ExternalInput")
    out_d = nc.dram_tensor("out", [N_EX, N, K], F32, kind="ExternalOutput")

    with tile.TileContext(nc) as tc, ExitStack() as ctx:
        const_pool = ctx.enter_context(tc.tile_pool(name="consts", bufs=1))
        x_pool = ctx.enter_context(tc.tile_pool(name="x", bufs=4))
        xT_pool = ctx.enter_context(tc.tile_pool(name="xT", bufs=3))
        c_pool = ctx.enter_context(tc.tile_pool(name="c", bufs=4))
        small_pool = ctx.enter_context(tc.tile_pool(name="small", bufs=4))

        ps_t = ctx.enter_context(tc.tile_pool(name="ps_t", bufs=2, space="PSUM"))
        ps_b = ctx.enter_context(tc.tile_pool(name="ps_b", bufs=2, space="PSUM"))
        ps_m = ctx.enter_context(tc.tile_pool(name="ps_m", bufs=4, space="PSUM"))

        # ---- constants ----
        Wt = const_pool.tile([128, 2, M], BF16, tag="Wt")
        nc.sync.dma_start(Wt[:], Wt_d[:])
        WtT = const_pool.tile([128, 2, D], BF16, tag="WtT")
        nc.sync.dma_start(WtT[:], WtT_d[:])
        id128 = const_pool.tile([128, 128], BF16, tag="id128")
        nc.sync.dma_start(id128[:], id128_d[:])
        ones128 = const_pool.tile([128, 1], BF16, tag="ones128")
        nc.sync.dma_start(ones128[:], ones_d[:])
        onesrow = const_pool.tile([1, 128], BF16, tag="onesrow")
        nc.sync.dma_start(onesrow[:], ones_d.ap().rearrange("p o -> o p"))
        bmask = const_pool.tile([N, M], BF16, tag="bmask")
        nc.sync.dma_start(bmask[:], bmask_d[:])
        bmaskT = const_pool.tile([128, 2, N], BF16, tag="bmaskT")
        nc.sync.dma_start(bmaskT[:], bmaskT_d[:])
        out_stage = const_pool.tile([N, N_EX, K], F32, tag="out_stage")
        eps_t = const_pool.tile([N, 1], F32, tag="eps")
        nc.vector.memset(eps_t[:], EPS)
        eps_r = const_pool.tile([1, 1], F32, tag="eps_r")
        nc.vector.memset(eps_r[:], EPS)

        # ---- x loads (prefetch, chunked so compute can start early) ----
        x_tiles = []
        for e in range(N_EX):
            xs = x_pool.tile([128, N_T, D], BF16, tag="x")
            for q in range(4):
                nc.sync.dma_start(xs[:, 8 * q:8 * (q + 1), :],
                                  x_d[e, :, 8 * q:8 * (q + 1), :])
            x_tiles.append(xs)

        st = [dict() for _ in range(N_EX)]

        # one shared PSUM bank subdivided for the small per-round tiles.
        # layout (f32 columns): yT @0..32, accT @32..64, rbc @64..96,
        # wsps @96..128, nrow @128..144, acc(j2 only) @224..480 (parts 0..15)
        def misc_views():
            m = ps_m.tile([128, 480], F32, tag="misc")
            yT_ps = m[:, 0:32].rearrange("p (c n) -> p c n", n=N)
            accT_ps = m[:, 32:64].rearrange("p (c n) -> p c n", n=N)
            rbc_ps = m[:, 64:96].rearrange("p (c n) -> p c n", n=N)
            wsps = m[:, 96:128].rearrange("p (c n) -> p c n", n=N)
            nrow_ps = m[0:1, 128:144]
            acc_ps = m[0:N, 224:480]
            return yT_ps, accT_ps, rbc_ps, wsps, nrow_ps, acc_ps

        # big-copy engine rotation: P7 V5 A4 per 16
        cp_engines = [nc.gpsimd, nc.vector, nc.scalar, nc.gpsimd,
                      nc.vector, nc.gpsimd, nc.scalar, nc.vector,
                      nc.gpsimd, nc.gpsimd, nc.vector, nc.scalar,
                      nc.gpsimd, nc.vector, nc.gpsimd, nc.scalar]

        def phase_T(e):
            """Build xT[d, i] via PE transposes; fold in colsum matmuls
            (round-0 shortcut: c uniform -> yT0 = colsum(x)/16)."""
            xs = x_tiles[e]
            xT = xT_pool.tile([128, 2, IN], BF16, tag="xT")
            cs = misc_views()
            cs_ps = cs[0]  # yT slot of this misc buf
            for tp in range(N_T // 2):
                psT = ps_t.tile([128, 2, 2, 128], BF16, tag="psT")
                for ti in range(2):
                    t = 2 * tp + ti
                    for dc in range(2):
                        nc.tensor.transpose(
                            psT[:, dc, ti, :],
                            xs[:, t, 128 * dc:128 * (dc + 1)], id128[:])
                        nc.tensor.matmul(
                            cs_ps[:, dc, 0:1],
                            xs[:, t, 128 * dc:128 * (dc + 1)], ones128[:],
                            start=(t == 0), stop=(t == N_T - 1),
                            skip_group_check=True)
                eng = cp_engines[tp]
                dst = xT[:, :, 256 * tp:256 * (tp + 1)]
                if eng is nc.vector:
                    eng.tensor_copy(dst, psT[:])
                elif eng is nc.scalar:
                    eng.copy(dst.bitcast(F32), psT[:].bitcast(F32))
                else:
                    eng.tensor_copy(dst.bitcast(F32), psT[:].bitcast(F32))
            st[e]["xT"] = xT
            st[e]["cs_ps"] = cs_ps

        def phase_J(e, j):
            """One routing round: yT -> accT -> squash -> WS -> b ->
            softmax (j<2) or final output (j=2)."""
            xT = st[e]["xT"]
            xs = x_tiles[e]
            c_all = st[e].get("c_all")
            yT_ps, accT_ps, rbc_ps, wsps, nrow_ps, acc_ps = misc_views()

            # --- yT[d, n] = sum_i x[i, d] c[i, n] ---
            if j == 0:
                yT_sb = small_pool.tile([128, 2, N], BF16, tag="yTsb")
                nc.vector.tensor_scalar_mul(
                    yT_sb[:],
                    st[e]["cs_ps"][:, :, 0:1].to_broadcast([128, 2, N]),
                    1.0 / N)
            else:
                for t in range(N_T):
                    for dc in range(2):
                        nc.tensor.matmul(
                            yT_ps[:, dc, :],
                            xs[:, t, 128 * dc:128 * (dc + 1)], c_all[:, t, :],
                            start=(t == 0), stop=(t == N_T - 1),
                            skip_group_check=True)
                yT_sb = small_pool.tile([128, 2, N], BF16, tag="yTsb")
                nc.gpsimd.tensor_copy(yT_sb[:], yT_ps[:])

            if j == 2:
                # final round: plain acc [16, 256], compact extraction
                for dc in range(2):
                    nc.tensor.matmul(acc_ps[:], yT_sb[:, dc, :], Wt[:, dc, :],
                                     start=(dc == 0), stop=(dc == 1),
                                     skip_group_check=True)
                om = small_pool.tile([N, M], BF16, tag="om")
                nc.gpsimd.tensor_mul(om[:], acc_ps[:], bmask[:])
                sq = small_pool.tile([N, M], BF16, tag="sq")
                nrm2 = small_pool.tile([N, 1], F32, tag="nrm2")
                nc.scalar.activation(sq[:], om[:], AF.Square,
                                     accum_out=nrm2[:])
                lnv = small_pool.tile([N, 1], F32, tag="lnv")
                nc.scalar.activation(lnv[:], nrm2[:], AF.Ln, bias=eps_t[:])
                rinv = small_pool.tile([N, 1], F32, tag="rinv")
                nc.scalar.activation(rinv[:], lnv[:], AF.Exp, scale=-0.5)
                o_c = small_pool.tile([N, K], F32, tag="o_c")
                nc.vector.tensor_reduce(
                    o_c[:], om[:].rearrange("p (g k) -> p k g", k=K),
                    axis=mybir.AxisListType.X, op=mybir.AluOpType.add)
                nc.vector.tensor_scalar_mul(out_stage[:, e, :], o_c[:],
                                            rinv[:])
                return

            # --- accT[m, n] = sum_d W[d, m] yT[d, n] (transposed form) ---
            for mc in range(2):
                for dc in range(2):
                    nc.tensor.matmul(
                        accT_ps[:, mc, :],
                        Wt[:, dc, 128 * mc:128 * (mc + 1)], yT_sb[:, dc, :],
                        start=(dc == 0), stop=(dc == 1),
                        skip_group_check=True)
            omT = small_pool.tile([128, 2, N], BF16, tag="omT")
            nc.gpsimd.tensor_mul(omT[:], accT_ps[:], bmaskT[:])

            # branch 1 (rinv): sqT -> nrow -> ln -> exp -> rbc
            sqT = small_pool.tile([128, 2, N], BF16, tag="sqT")
            nc.vector.tensor_mul(sqT[:], omT[:], omT[:])
            for mc in range(2):
                nc.tensor.matmul(nrow_ps[:], ones128[:], sqT[:, mc, :],
                                 start=(mc == 0), stop=(mc == 1),
                                 skip_group_check=True)
            lnr = small_pool.tile([1, N], F32, tag="lnr")
            nc.scalar.activation(lnr[:], nrow_ps[:], AF.Ln, bias=eps_r[:])
            rrow = small_pool.tile([1, N], BF16, tag="rrow")
            nc.scalar.activation(rrow[:], lnr[:], AF.Exp, scale=-0.5)
            for mc in range(2):
                nc.tensor.matmul(rbc_ps[:, mc, :], onesrow[:], rrow[:],
                                 start=True, stop=True,
                                 skip_group_check=True)

            # branch 2 (runs in parallel): WS_raw = W @ omT; rinv scales
            # out of the m-contraction, applied at the ws copy
            for dc in range(2):
                for mc in range(2):
                    nc.tensor.matmul(
                        wsps[:, dc, :],
                        WtT[:, mc, 128 * dc:128 * (dc + 1)], omT[:, mc, :],
                        start=(mc == 0), stop=(mc == 1),
                        skip_group_check=True)
            ws = small_pool.tile([128, 2, N], BF16, tag="ws")
            nc.vector.tensor_mul(ws[:], wsps[:], rbc_ps[:])

            # --- b = x @ WS ---
            b_ps = ps_b.tile([128, N_T, N], F32, tag="b")
            for t in range(N_T):
                for dc in range(2):
                    nc.tensor.matmul(
                        b_ps[:, t, :],
                        xT[:, dc, 128 * t:128 * (t + 1)], ws[:, dc, :],
                        start=(dc == 0), stop=(dc == 1),
                        skip_group_check=True)

            # --- softmax over n, pipelined in i-halves so the next round's
            # yT matmuls for tiles 0..15 start after only half the tail ---
            e_all = c_pool.tile([128, N_T, N], BF16, tag="e_all")
            s_sum = c_pool.tile([128, N_T], F32, tag="s_sum")
            s_r = c_pool.tile([128, N_T], F32, tag="s_r")
            c_new = c_pool.tile([128, N_T, N], BF16, tag="c_all")
            H = N_T // 2
            for h in range(2):
                sl = slice(H * h, H * (h + 1))
                nc.scalar.activation(e_all[:, sl, :], b_ps[:, sl, :], AF.Exp)
                nc.vector.tensor_reduce(s_sum[:, sl], e_all[:, sl, :],
                                        axis=mybir.AxisListType.X,
                                        op=mybir.AluOpType.add)
                nc.vector.reciprocal(s_r[:, sl], s_sum[:, sl])
                nc.vector.tensor_mul(
                    c_new[:, sl, :], e_all[:, sl, :],
                    s_r[:, sl].to_broadcast([128, H, N]))
            st[e]["c_all"] = c_new

        # wavefront over examples to keep PE dense across dependency stalls
        order = [(0, "T"), (0, 0), (1, "T"), (0, 1), (1, 0), (2, "T"),
                 (0, 2), (1, 1), (2, 0), (3, "T"), (1, 2), (2, 1), (3, 0),
                 (2, 2), (3, 1), (3, 2)]
        for e, ph in order:
            if ph == "T":
                phase_T(e)
            else:
                phase_J(e, ph)

        nc.sync.dma_start(out_d.ap().rearrange("e n k -> n e k"), out_stage[:])

    nc.compile()
    return nc


_NC_CACHE = {}


def _get_nc():
    if "nc" not in _NC_CACHE:
        _NC_CACHE["nc"] = build_kernel()
    return _NC_CACHE["nc"]


def make_const_inputs():
    bf = ml_dtypes.bfloat16
    id128 = np.eye(128, dtype=bf)
    ones128 = np.ones((128, 1), dtype=bf)
    bmask = np.zeros((N, M), dtype=np.float32)
    for n in range(N):
        bmask[n, n * K:(n + 1) * K] = 1.0
    # bmaskT[p, mc, n] = bmask[n, mc*128 + p]
    bmaskT = np.ascontiguousarray(
        bmask.T.reshape(2, 128, N).transpose(1, 0, 2))
    return id128, ones128, bmask.astype(bf), bmaskT.astype(bf)


def kernel(x, W, num_capsule=None, dim_capsule=None, routings=None, **_):
    bf = ml_dtypes.bfloat16
    x = np.asarray(x, dtype=np.float32)
    W = np.asarray(W, dtype=np.float32)
    assert x.shape == (B, IN, D), x.shape

    nc = _get_nc()
    id128, ones128, bmask, bmaskT = make_const_inputs()
    W0 = W[0]
    Wt = np.ascontiguousarray(
        W0.reshape(2, 128, M).transpose(1, 0, 2)).astype(bf)
    WtT = np.ascontiguousarray(
        W0.T.reshape(2, 128, D).transpose(1, 0, 2)).astype(bf)

    # x[b, i, d] -> [core, e, p, t, d] with i = t*128 + p
    xr = np.ascontiguousarray(
        x.reshape(N_CORES, N_EX, N_T, 128, D).transpose(0, 1, 3, 2, 4)
    ).astype(bf)

    in_maps = []
    for c in range(N_CORES):
        in_maps.append({"x": xr[c], "Wt": Wt, "WtT": WtT, "id128": id128,
                        "ones128": ones128, "bmask": bmask,
                        "bmaskT": bmaskT})

    res = run_bass_kernel_spmd(nc, in_maps, core_ids=list(range(N_CORES)))
    out = np.concatenate([r["out"] for r in res.results], axis=0)
    return out.astype(np.float32)
